# revision 1
# baseline (speedup 1.0000x reference)
"""Bidirectional LSTM (TF BasicLSTMCell semantics) on 8 Trainium2 NeuronCores.

Problem: x [64, 128, 512], per-direction W [1024, 2048], b [2048].
out [64, 128, 1024] = concat(h_fw, h_bw) over a T=128 sequential scan.

Sharding: 2 (direction) x 4 (batch quarters) = 8 cores, B_local = 16.
Every core runs the SAME program; direction is handled host-side by
time-reversing x (and the returned outputs) for the backward cores and
binding W_bw instead of W_fw.

The scan is weight-load bound: each step must stream all 64 fp16 128x128
Whh tiles through the PE's LDWEIGHTS port (~53ns each with FWL), while the
matmuls themselves only move 16 batch columns.  Everything else is
scheduled to hide under that ~3.4us/step weight stream:

  phase 1:  G^T = Wx^T x^T + b computed in 32 units of (512 cols x 2
            m-tiles).  8 units run before the scan; the rest interleave
            one unit per two scan steps, their 512-col matmuls streaming
            on the PE's moving-operand port while the scan owns the
            weight port.
  scan:     hidden dim split in two halves pipelined against each other
            (half-0's gate math runs under half-1's matmuls and vice
            versa).  Gates are packed two-per-PSUM-bank (j,i / f,o) so
            consecutive matmuls alternate banks and z-assembly is 2 DVE
            ops; j uses a real Tanh (zones j|i|f|o via host-side gate
            permutation); the c-update multiply runs on GpSimd.
  output:   h stored hidden-major [512, T*B] fp32 in SBUF, DMA'd out in
            16-step chunks as the scan produces them; host does the final
            transpose in numpy.
"""

import os
import sys

import numpy as np

for _p in ("/opt/trn_rl_repo", "/root/.axon_site/_ro/trn_rl_repo"):
    if os.path.isdir(_p) and _p not in sys.path:
        sys.path.insert(0, _p)

from contextlib import ExitStack

import concourse.bass as bass
import concourse.mybir as mybir
import concourse.tile as tile
from concourse import bacc

F32 = mybir.dt.float32
F16 = mybir.dt.float16
AF = mybir.ActivationFunctionType

B_FULL = 64
B_LOC = 16  # batch per core
T = 128
F = 512
H = 512
NG = 4 * H  # 2048 gate columns
KT = 4      # 128-row contraction tiles over F or H
MT = 16     # 128-col gate tiles
FORGET_BIAS = 1.0
P1_W = 512          # phase-1 column-group width
P1_UPFRONT = 8      # phase-1 units emitted before the scan
P1_STRIDE = 3       # scan steps between interleaved phase-1 units
DMA_CHUNK = 16      # scan steps per output DMA chunk


def build_nc(t_steps: int = T, repeat: int = 1, scan_mm: bool = True,
             scan_math: bool = True, p1_interleave: bool = True) -> bass.Bass:
    tb = t_steps * B_LOC

    nc = bacc.Bacc("TRN2", target_bir_lowering=False, debug=False)
    x_d = nc.dram_tensor("xT", [F, tb], F16, kind="ExternalInput").ap()
    wx_d = nc.dram_tensor("wx", [F, NG], F16, kind="ExternalInput").ap()
    whh_d = nc.dram_tensor("whh", [H, NG], F16, kind="ExternalInput").ap()
    bias_d = nc.dram_tensor("bias", [128, MT], F32, kind="ExternalInput").ap()
    ident_d = nc.dram_tensor("ident", [128, 128], F16, kind="ExternalInput").ap()
    y_d = nc.dram_tensor("y", [H, tb], F32, kind="ExternalOutput").ap()

    with ExitStack() as ctx:
        tc = ctx.enter_context(tile.TileContext(nc))
        const = ctx.enter_context(tc.tile_pool(name="const", bufs=1))
        wx_sb = const.tile([128, KT * NG], F16, tag="wx")     # col = k*NG + m*128 + j
        whh_sb = const.tile([128, KT * NG], F16, tag="whh")   # col = k*NG + m*128 + j
        g_sb = const.tile([128, MT * tb], F16, tag="g")       # col = m*tb + t*16 + b
        hall = const.tile([128, KT * tb], F32, tag="hall")    # col = k*tb + t*16 + b
        xT_sb = const.tile([128, KT * tb], F16, tag="xT")     # col = k*tb + (t,b)
        bias_sb = const.tile([128, MT], F32, tag="bias")
        c_sb = const.tile([128, KT * B_LOC], F32, tag="c")    # col = (mi, b)
        ident_sb = const.tile([128, 128], F16, tag="ident")
        zsrc = None
        if not scan_mm:
            zsrc = const.tile([128, 4 * B_LOC], F32, tag="zsrc")

        for k in range(KT):
            nc.sync.dma_start(wx_sb[:, k * NG:(k + 1) * NG], wx_d[k * 128:(k + 1) * 128, :])
            nc.sync.dma_start(whh_sb[:, k * NG:(k + 1) * NG], whh_d[k * 128:(k + 1) * 128, :])
            nc.sync.dma_start(xT_sb[:, k * tb:(k + 1) * tb], x_d[k * 128:(k + 1) * 128, :])
        nc.sync.dma_start(bias_sb[:], bias_d[:, :])
        nc.sync.dma_start(ident_sb[:], ident_d[:, :])

        if repeat > 1:
            loop_cm = tc.For_i(0, repeat, 1)
            loop_cm.__enter__()

        nc.vector.memset(c_sb[:], 0.0)
        if zsrc is not None:
            nc.vector.memset(zsrc[:], 0.0)

        g4 = g_sb[:].rearrange("p (g mi c) -> p g mi c", g=4, mi=4)
        h3 = hall[:].rearrange("p (k c) -> p k c", k=KT)

        # ---------- phase 1 units: G^T[m*128+p, (t,b)] = sum_f Wx[f, gate] x[(t,b), f] + b
        # unit u = (cc, mp): 512 cols x m-tiles (2mp, 2mp+1), two PSUM banks
        # interleaved so consecutive matmuls alternate banks.
        p1_w = min(P1_W, tb)
        n_cc = tb // p1_w
        p1_units = [(cc, mp) for cc in range(n_cc) for mp in range(MT // 2)]
        p1_p = ctx.enter_context(tc.tile_pool(name="p1", bufs=1, space="PSUM"))

        def emit_p1_unit(u):
            cc, mp = p1_units[u]
            ms = (2 * mp, 2 * mp + 1)
            pst = [p1_p.tile([128, p1_w], F32, tag="p1%d" % i, name="p1%d" % i)
                   for i in range(2)]
            for k in range(KT):
                for i, m in enumerate(ms):
                    nc.tensor.matmul(
                        pst[i][:],
                        wx_sb[:, k * NG + m * 128: k * NG + (m + 1) * 128],
                        xT_sb[:, k * tb + cc * p1_w: k * tb + (cc + 1) * p1_w],
                        start=(k == 0), stop=(k == KT - 1),
                    )
            for i, m in enumerate(ms):
                nc.vector.tensor_scalar_add(
                    g_sb[:, m * tb + cc * p1_w: m * tb + (cc + 1) * p1_w],
                    pst[i][:],
                    bias_sb[:, m:m + 1],
                )

        n_upfront = P1_UPFRONT if p1_interleave else len(p1_units)
        for u in range(n_upfront):
            emit_p1_unit(u)
        p1_next = n_upfront

        # ---------- scan
        # Gate order is host-permuted to j|i|f|o.  Per half Hh (hidden units
        # [Hh*256, Hh*256+256)): PSUM tile A holds gates (j,i), B holds (f,o),
        # each [128, (gslot:2)(mr:2)(b:16)] in its own bank.
        with tc.tile_pool(name="sps", bufs=1, space="PSUM") as sps_p, \
             tc.tile_pool(name="zt", bufs=3) as z_p, \
             tc.tile_pool(name="act", bufs=3) as a_p, \
             tc.tile_pool(name="h16", bufs=3) as h_p, \
             tc.tile_pool(name="tmp", bufs=3) as tmp_p:
            hq = [None] * KT
            for Hh in (0, 1):
                t0 = h_p.tile([128, 2 * B_LOC], F16, tag="h16_%d" % Hh,
                              name="h0_%d" % Hh)
                nc.vector.memset(t0[:], 0.0)
                hq[2 * Hh] = t0[:, 0:B_LOC]
                hq[2 * Hh + 1] = t0[:, B_LOC:2 * B_LOC]

            # matmul emission order per k: A,B,A,B (bank alternation)
            MM_ORDER = [(0, 0), (2, 0), (1, 0), (3, 0),
                        (0, 1), (2, 1), (1, 1), (3, 1)]
            for t in range(t_steps):
                ts_ = slice(t * B_LOC, (t + 1) * B_LOC)
                # --- all 64 matmuls first (both halves read step t-1's h)
                ps = {}
                for Hh in (0, 1) if scan_mm else ():
                    mlo = 2 * Hh
                    for nm, tag in (("A", "zA"), ("B", "zB")):
                        ps[(Hh, nm)] = sps_p.tile(
                            [128, 4 * B_LOC], F32,
                            tag="%s%d" % (tag, Hh), name="%s%d" % (tag, Hh))
                    # G-add: start matmul clears the bank and writes G (identity
                    # weights); the h-matmuls then accumulate z on top.
                    for half, nm in ((0, "A"), (1, "B")):
                        nc.tensor.matmul(
                            ps[(Hh, nm)][:].rearrange(
                                "p (g mr c) -> p g mr c", g=2, mr=2),
                            ident_sb[:],
                            g4[:, 2 * half:2 * half + 2, mlo:mlo + 2, ts_],
                            start=True, stop=False,
                            skip_group_check=True,
                        )
                    for k in range(KT):
                        for g, mr in MM_ORDER:
                            tl = ps[(Hh, "A")] if g < 2 else ps[(Hh, "B")]
                            slot = g % 2
                            m = 4 * g + mlo + mr
                            nc.tensor.matmul(
                                tl[:, slot * 2 * B_LOC + mr * B_LOC:
                                   slot * 2 * B_LOC + (mr + 1) * B_LOC],
                                whh_sb[:, k * NG + m * 128: k * NG + (m + 1) * 128],
                                hq[k],
                                start=False,
                                stop=(k == KT - 1 and (g, mr) == ((1, 1) if g < 2 else (3, 1))),
                                skip_group_check=True,
                            )
                # --- gate math per half (half 0 runs under half 1's matmuls)
                for Hh in (0, 1) if scan_math else ():
                    mlo = 2 * Hh
                    # sio zones j2x|i|f|o; tanh(z_j) = 2*sig(2 z_j) - 1 with the
                    # 2x folded into the host-side j weights/bias.
                    sio = a_p.tile([128, 4 * 2 * B_LOC], F32, tag="sio%d" % Hh)
                    for half, nm in ((0, "A"), (1, "B")):
                        nc.scalar.activation(
                            sio[:, half * 64:(half + 1) * 64],
                            (ps[(Hh, nm)] if scan_mm else zsrc)[:],
                            AF.Sigmoid)
                    # sig(i)*tanh(j) = 2*sig(i)*(sig(2z_j) - 0.5), the 2x
                    # folded into the c-accumulate below (both fused STT ops).
                    ch = c_sb[:, Hh * 32:(Hh + 1) * 32]
                    tmp = tmp_p.tile([128, 2 * B_LOC], F32, tag="tmp%d" % Hh)
                    nc.vector.scalar_tensor_tensor(
                        tmp[:], sio[:, 0:32], 0.5, sio[:, 32:64],
                        mybir.AluOpType.subtract, mybir.AluOpType.mult)
                    nc.vector.tensor_mul(ch, ch, sio[:, 64:96])       # c *= sig(f)
                    nc.vector.scalar_tensor_tensor(
                        ch, tmp[:], 2.0, ch,
                        mybir.AluOpType.mult, mybir.AluOpType.add)
                    tanc = a_p.tile([128, 2 * B_LOC], F32, tag="tanc%d" % Hh)
                    nc.scalar.activation(tanc[:], ch, AF.Tanh)

                    # fp16 h for the next step's matmuls first (critical path),
                    # then the fp32 output copy on GpSimd (off both hot engines)
                    hnew = h_p.tile([128, 2 * B_LOC], F16, tag="h16_%d" % Hh,
                                    name="hnew%d" % Hh)
                    nc.vector.tensor_mul(hnew[:], tanc[:], sio[:, 96:128])
                    hq[mlo] = hnew[:, 0:B_LOC]
                    hq[mlo + 1] = hnew[:, B_LOC:2 * B_LOC]
                    hv = h3[:, mlo:mlo + 2, ts_]
                    nc.gpsimd.tensor_mul(
                        hv,
                        tanc[:].rearrange("p (m c) -> p m c", m=2),
                        sio[:, 96:128].rearrange("p (m c) -> p m c", m=2),
                    )

                # interleave the remaining phase-1 units
                if t % P1_STRIDE == 0 and p1_next < len(p1_units):
                    emit_p1_unit(p1_next)
                    p1_next += 1

                # stream finished output chunks out while the scan runs
                if scan_math and (t + 1) % DMA_CHUNK == 0:
                    ci = (t + 1) // DMA_CHUNK - 1
                    w = DMA_CHUNK * B_LOC
                    for k in range(KT):
                        nc.sync.dma_start(
                            y_d[k * 128:(k + 1) * 128, ci * w:(ci + 1) * w],
                            hall[:, k * tb + ci * w: k * tb + (ci + 1) * w])
            while p1_next < len(p1_units):
                emit_p1_unit(p1_next)
                p1_next += 1
            # tail of the output if t_steps isn't a DMA_CHUNK multiple
            rem = t_steps % DMA_CHUNK
            if rem and scan_math:
                c0 = (t_steps - rem) * B_LOC
                for k in range(KT):
                    nc.sync.dma_start(
                        y_d[k * 128:(k + 1) * 128, c0:tb],
                        hall[:, k * tb + c0: k * tb + tb])

        if repeat > 1:
            loop_cm.__exit__(None, None, None)

    nc.compile()
    return nc


_BUILT: bass.Bass | None = None


def _get_built() -> bass.Bass:
    global _BUILT
    if _BUILT is None:
        _BUILT = build_nc(T)
    return _BUILT


def _permute_gates(W):
    """Reorder gate blocks (i,j,f,o) -> (j,i,f,o) along the last axis."""
    return np.concatenate(
        [W[..., H:2 * H], W[..., 0:H], W[..., 2 * H:3 * H], W[..., 3 * H:4 * H]],
        axis=-1)


def make_in_maps(x, W_fw, b_fw, W_bw, b_bw, t_steps: int = T):
    x = np.asarray(x, np.float32)
    in_maps = []
    for d, (Wd, bd) in enumerate(((W_fw, b_fw), (W_bw, b_bw))):
        Wd = _permute_gates(np.asarray(Wd, np.float32))
        bv = _permute_gates(np.asarray(bd, np.float32).copy())
        Wd[:, 0:H] *= 2.0   # j-gate: tanh(z) = 2*sigmoid(2z) - 1 on device
        bv[0:H] *= 2.0
        bv[2 * H:3 * H] += FORGET_BIAS  # fold forget bias into the f-gate bias
        wx = np.ascontiguousarray(Wd[:F]).astype(np.float16)
        whh = np.ascontiguousarray(Wd[F:]).astype(np.float16)
        bias = np.ascontiguousarray(bv.reshape(MT, 128).T)
        for g in range(4):
            xg = x[g * B_LOC:(g + 1) * B_LOC, :t_steps]
            if d == 1:
                xg = xg[:, ::-1, :]
            x_t = np.ascontiguousarray(
                xg.transpose(1, 0, 2).reshape(t_steps * B_LOC, F).T
            ).astype(np.float16)
            in_maps.append({"xT": x_t, "wx": wx, "whh": whh, "bias": bias,
                            "ident": np.eye(128, dtype=np.float16)})
    return in_maps


def assemble_out(results, t_steps: int = T):
    out = np.empty((B_FULL, t_steps, 2 * H), np.float32)
    for idx, r in enumerate(results):
        d, g = divmod(idx, 4)
        h = r["y"].reshape(H, t_steps, B_LOC).transpose(2, 1, 0)  # [16, T, 512]
        if d == 1:
            h = h[:, ::-1, :]
        out[g * B_LOC:(g + 1) * B_LOC, :, d * H:(d + 1) * H] = h
    return out


def kernel(x, W_fw, b_fw, W_bw, b_bw):
    from concourse.bass_utils import run_bass_kernel_spmd

    nc = _get_built()
    in_maps = make_in_maps(x, W_fw, b_fw, W_bw, b_bw)
    res = run_bass_kernel_spmd(nc, in_maps, core_ids=list(range(8)))
    return assemble_out(res.results)



# revision 4
# speedup vs baseline: 1.0665x; 1.0665x over previous
"""Bidirectional LSTM (TF BasicLSTMCell semantics) on 8 Trainium2 NeuronCores.

Problem: x [64, 128, 512], per-direction W [1024, 2048], b [2048].
out [64, 128, 1024] = concat(h_fw, h_bw) over a T=128 sequential scan.

Sharding: 2 (direction) x 4 (batch quarters) = 8 cores, B_local = 16.
Every core runs the SAME program; direction is handled host-side by
time-reversing x (and the returned outputs) for the backward cores and
binding W_bw instead of W_fw.

The scan is weight-load bound: each step must stream all 64 fp16 128x128
Whh tiles through the PE's LDWEIGHTS port (~53ns each with FWL), while the
matmuls themselves only move 16 batch columns.  Everything else is
scheduled to hide under that ~3.4us/step weight stream:

  phase 1:  G^T = Wx^T x^T + b computed in 32 units of (512 cols x 2
            m-tiles).  8 units run before the scan; the rest interleave
            one unit per two scan steps, their 512-col matmuls streaming
            on the PE's moving-operand port while the scan owns the
            weight port.
  scan:     hidden dim split in two halves pipelined against each other
            (half-0's gate math runs under half-1's matmuls and vice
            versa).  Gates are packed two-per-PSUM-bank (j,i / f,o) so
            consecutive matmuls alternate banks and z-assembly is 2 DVE
            ops; j uses a real Tanh (zones j|i|f|o via host-side gate
            permutation); the c-update multiply runs on GpSimd.
  output:   h stored hidden-major [512, T*B] fp32 in SBUF, DMA'd out in
            16-step chunks as the scan produces them; host does the final
            transpose in numpy.
"""

import os
import sys

import numpy as np

for _p in ("/opt/trn_rl_repo", "/root/.axon_site/_ro/trn_rl_repo"):
    if os.path.isdir(_p) and _p not in sys.path:
        sys.path.insert(0, _p)

from contextlib import ExitStack

import concourse.bass as bass
import concourse.mybir as mybir
import concourse.tile as tile
from concourse import bacc

F32 = mybir.dt.float32
F16 = mybir.dt.float16
AF = mybir.ActivationFunctionType

B_FULL = 64
B_LOC = 16  # batch per core
T = 128
F = 512
H = 512
NG = 4 * H  # 2048 gate columns
KT = 4      # 128-row contraction tiles over F or H
MT = 16     # 128-col gate tiles
FORGET_BIAS = 1.0
P1_W = 512          # phase-1 column-group width
P1_UPFRONT = 8      # phase-1 units emitted before the scan
P1_STRIDE = 4       # scan steps between interleaved phase-1 units
DMA_CHUNK = 16      # scan steps per output DMA chunk


def build_nc(t_steps: int = T, repeat: int = 1, scan_mm: bool = True,
             scan_math: bool = True, p1_interleave: bool = True) -> bass.Bass:
    tb = t_steps * B_LOC

    nc = bacc.Bacc("TRN2", target_bir_lowering=False, debug=False)
    x_d = nc.dram_tensor("xT", [F, tb], F16, kind="ExternalInput").ap()
    wx_d = nc.dram_tensor("wx", [F, NG], F16, kind="ExternalInput").ap()
    whh_d = nc.dram_tensor("whh", [H, NG], F16, kind="ExternalInput").ap()
    bias_d = nc.dram_tensor("bias", [128, MT], F32, kind="ExternalInput").ap()
    ident_d = nc.dram_tensor("ident", [128, 128], F16, kind="ExternalInput").ap()
    y_d = nc.dram_tensor("y", [H, tb], F32, kind="ExternalOutput").ap()

    with ExitStack() as ctx:
        tc = ctx.enter_context(tile.TileContext(nc))
        const = ctx.enter_context(tc.tile_pool(name="const", bufs=1))
        wx_sb = const.tile([128, KT * NG], F16, tag="wx")     # col = k*NG + m*128 + j
        whh_sb = const.tile([128, KT * NG], F16, tag="whh")   # col = k*NG + m*128 + j
        g_sb = const.tile([128, MT * tb], F16, tag="g")       # col = m*tb + t*16 + b
        hall = const.tile([128, KT * tb], F32, tag="hall")    # col = k*tb + t*16 + b
        xT_sb = const.tile([128, KT * tb], F16, tag="xT")     # col = k*tb + (t,b)
        bias_sb = const.tile([128, MT], F32, tag="bias")
        c_sb = const.tile([128, KT * B_LOC], F32, tag="c")    # col = (mi, b)
        ident_sb = const.tile([128, 128], F16, tag="ident")
        zsrc = None
        if not scan_mm:
            zsrc = const.tile([128, 8 * B_LOC], F32, tag="zsrc")

        for k in range(KT):
            nc.sync.dma_start(wx_sb[:, k * NG:(k + 1) * NG], wx_d[k * 128:(k + 1) * 128, :])
            nc.sync.dma_start(whh_sb[:, k * NG:(k + 1) * NG], whh_d[k * 128:(k + 1) * 128, :])
            nc.sync.dma_start(xT_sb[:, k * tb:(k + 1) * tb], x_d[k * 128:(k + 1) * 128, :])
        nc.sync.dma_start(bias_sb[:], bias_d[:, :])
        nc.sync.dma_start(ident_sb[:], ident_d[:, :])

        if repeat > 1:
            loop_cm = tc.For_i(0, repeat, 1)
            loop_cm.__enter__()

        nc.vector.memset(c_sb[:], 0.0)
        if zsrc is not None:
            nc.vector.memset(zsrc[:], 0.0)

        g4 = g_sb[:].rearrange("p (g mi c) -> p g mi c", g=4, mi=4)
        h3 = hall[:].rearrange("p (k c) -> p k c", k=KT)

        # ---------- phase 1 units: G^T[m*128+p, (t,b)] = sum_f Wx[f, gate] x[(t,b), f] + b
        # unit u = (cc, mp): 512 cols x m-tiles (2mp, 2mp+1), two PSUM banks
        # interleaved so consecutive matmuls alternate banks.
        p1_w = min(P1_W, tb)
        n_cc = tb // p1_w
        p1_units = [(cc, mp) for cc in range(n_cc) for mp in range(MT // 2)]
        p1_p = ctx.enter_context(tc.tile_pool(name="p1", bufs=1, space="PSUM"))

        def emit_p1_unit(u):
            cc, mp = p1_units[u]
            ms = (2 * mp, 2 * mp + 1)
            pst = [p1_p.tile([128, p1_w], F32, tag="p1%d" % i, name="p1%d" % i)
                   for i in range(2)]
            for k in range(KT):
                for i, m in enumerate(ms):
                    nc.tensor.matmul(
                        pst[i][:],
                        wx_sb[:, k * NG + m * 128: k * NG + (m + 1) * 128],
                        xT_sb[:, k * tb + cc * p1_w: k * tb + (cc + 1) * p1_w],
                        start=(k == 0), stop=(k == KT - 1),
                    )
            for i, m in enumerate(ms):
                nc.vector.tensor_scalar_add(
                    g_sb[:, m * tb + cc * p1_w: m * tb + (cc + 1) * p1_w],
                    pst[i][:],
                    bias_sb[:, m:m + 1],
                )

        n_upfront = P1_UPFRONT if p1_interleave else len(p1_units)
        for u in range(n_upfront):
            emit_p1_unit(u)
        p1_next = n_upfront

        # ---------- scan
        # Gate order is host-permuted to j|i|f|o.  Per half Hh (hidden units
        # [Hh*256, Hh*256+256)): PSUM tile A holds gates (j,i), B holds (f,o),
        # each [128, (gslot:2)(mr:2)(b:16)] in its own bank.
        with tc.tile_pool(name="sps", bufs=1, space="PSUM") as sps_p, \
             tc.tile_pool(name="zt", bufs=3) as z_p, \
             tc.tile_pool(name="act", bufs=3) as a_p, \
             tc.tile_pool(name="h16", bufs=3) as h_p, \
             tc.tile_pool(name="tmp", bufs=3) as tmp_p:
            hq = [None] * KT
            for Hh in (0, 1):
                t0 = h_p.tile([128, 2 * B_LOC], F16, tag="h16_%d" % Hh,
                              name="h0_%d" % Hh)
                nc.vector.memset(t0[:], 0.0)
                hq[2 * Hh] = t0[:, 0:B_LOC]
                hq[2 * Hh + 1] = t0[:, B_LOC:2 * B_LOC]

            for t in range(t_steps):
                ts_ = slice(t * B_LOC, (t + 1) * B_LOC)
                # --- all 64 matmuls first (both halves read step t-1's h).
                # One PSUM bank per half, [128, (g:4)(mr:2)(b:16)]: a single
                # G-add identity matmul opens the group (2 PE slots/step
                # saved vs the old 2-banks-per-half split), 32 h-matmuls
                # accumulate on top.
                ps = {}
                for Hh in (0, 1) if scan_mm else ():
                    mlo = 2 * Hh
                    ps[Hh] = sps_p.tile(
                        [128, 8 * B_LOC], F32,
                        tag="z%d" % Hh, name="z%d" % Hh)
                    nc.tensor.matmul(
                        ps[Hh][:].rearrange(
                            "p (g mr c) -> p g mr c", g=4, mr=2),
                        ident_sb[:],
                        g4[:, :, mlo:mlo + 2, ts_],
                        start=True, stop=False,
                        skip_group_check=True,
                    )
                    for k in range(KT):
                        for g in range(4):
                            for mr in range(2):
                                m = 4 * g + mlo + mr
                                nc.tensor.matmul(
                                    ps[Hh][:, g * 2 * B_LOC + mr * B_LOC:
                                       g * 2 * B_LOC + (mr + 1) * B_LOC],
                                    whh_sb[:, k * NG + m * 128:
                                           k * NG + (m + 1) * 128],
                                    hq[k],
                                    start=False,
                                    stop=(k == KT - 1 and g == 3 and mr == 1),
                                    skip_group_check=True,
                                )
                # --- gate math per half (half 0 runs under half 1's matmuls)
                for Hh in (0, 1) if scan_math else ():
                    mlo = 2 * Hh
                    # sio zones j2x|i|f|o; tanh(z_j) = 2*sig(2 z_j) - 1 with the
                    # 2x folded into the host-side j weights/bias.
                    sio = a_p.tile([128, 4 * 2 * B_LOC], F32, tag="sio%d" % Hh)
                    nc.scalar.activation(
                        sio[:],
                        (ps[Hh] if scan_mm else zsrc)[:],
                        AF.Sigmoid)
                    # sig(i)*tanh(j) = 2*sig(i)*(sig(2z_j) - 0.5), the 2x
                    # folded into the c-accumulate below (both fused STT ops).
                    ch = c_sb[:, Hh * 32:(Hh + 1) * 32]
                    tmp = tmp_p.tile([128, 2 * B_LOC], F32, tag="tmp%d" % Hh)
                    nc.vector.scalar_tensor_tensor(
                        tmp[:], sio[:, 0:32], 0.5, sio[:, 32:64],
                        mybir.AluOpType.subtract, mybir.AluOpType.mult)
                    nc.vector.tensor_mul(ch, ch, sio[:, 64:96])       # c *= sig(f)
                    nc.vector.scalar_tensor_tensor(
                        ch, tmp[:], 2.0, ch,
                        mybir.AluOpType.mult, mybir.AluOpType.add)
                    tanc = a_p.tile([128, 2 * B_LOC], F32, tag="tanc%d" % Hh)
                    nc.scalar.activation(tanc[:], ch, AF.Tanh)

                    # fp16 h for the next step's matmuls first (critical path),
                    # then the fp32 output copy on GpSimd (off both hot engines)
                    hnew = h_p.tile([128, 2 * B_LOC], F16, tag="h16_%d" % Hh,
                                    name="hnew%d" % Hh)
                    nc.vector.tensor_mul(hnew[:], tanc[:], sio[:, 96:128])
                    hq[mlo] = hnew[:, 0:B_LOC]
                    hq[mlo + 1] = hnew[:, B_LOC:2 * B_LOC]
                    hv = h3[:, mlo:mlo + 2, ts_]
                    nc.gpsimd.tensor_mul(
                        hv,
                        tanc[:].rearrange("p (m c) -> p m c", m=2),
                        sio[:, 96:128].rearrange("p (m c) -> p m c", m=2),
                    )

                # interleave the remaining phase-1 units
                if t % P1_STRIDE == 0 and p1_next < len(p1_units):
                    emit_p1_unit(p1_next)
                    p1_next += 1

                # stream finished output chunks out while the scan runs
                if scan_math and (t + 1) % DMA_CHUNK == 0:
                    ci = (t + 1) // DMA_CHUNK - 1
                    w = DMA_CHUNK * B_LOC
                    for k in range(KT):
                        nc.sync.dma_start(
                            y_d[k * 128:(k + 1) * 128, ci * w:(ci + 1) * w],
                            hall[:, k * tb + ci * w: k * tb + (ci + 1) * w])
            while p1_next < len(p1_units):
                emit_p1_unit(p1_next)
                p1_next += 1
            # tail of the output if t_steps isn't a DMA_CHUNK multiple
            rem = t_steps % DMA_CHUNK
            if rem and scan_math:
                c0 = (t_steps - rem) * B_LOC
                for k in range(KT):
                    nc.sync.dma_start(
                        y_d[k * 128:(k + 1) * 128, c0:tb],
                        hall[:, k * tb + c0: k * tb + tb])

        if repeat > 1:
            loop_cm.__exit__(None, None, None)

    nc.compile()
    return nc


_BUILT: bass.Bass | None = None


def _get_built() -> bass.Bass:
    global _BUILT
    if _BUILT is None:
        _BUILT = build_nc(T)
    return _BUILT


def _permute_gates(W):
    """Reorder gate blocks (i,j,f,o) -> (j,i,f,o) along the last axis."""
    return np.concatenate(
        [W[..., H:2 * H], W[..., 0:H], W[..., 2 * H:3 * H], W[..., 3 * H:4 * H]],
        axis=-1)


def make_in_maps(x, W_fw, b_fw, W_bw, b_bw, t_steps: int = T):
    x = np.asarray(x, np.float32)
    in_maps = []
    for d, (Wd, bd) in enumerate(((W_fw, b_fw), (W_bw, b_bw))):
        Wd = _permute_gates(np.asarray(Wd, np.float32))
        bv = _permute_gates(np.asarray(bd, np.float32).copy())
        Wd[:, 0:H] *= 2.0   # j-gate: tanh(z) = 2*sigmoid(2z) - 1 on device
        bv[0:H] *= 2.0
        bv[2 * H:3 * H] += FORGET_BIAS  # fold forget bias into the f-gate bias
        wx = np.ascontiguousarray(Wd[:F]).astype(np.float16)
        whh = np.ascontiguousarray(Wd[F:]).astype(np.float16)
        bias = np.ascontiguousarray(bv.reshape(MT, 128).T)
        for g in range(4):
            xg = x[g * B_LOC:(g + 1) * B_LOC, :t_steps]
            if d == 1:
                xg = xg[:, ::-1, :]
            x_t = np.ascontiguousarray(
                xg.transpose(1, 0, 2).reshape(t_steps * B_LOC, F).T
            ).astype(np.float16)
            in_maps.append({"xT": x_t, "wx": wx, "whh": whh, "bias": bias,
                            "ident": np.eye(128, dtype=np.float16)})
    return in_maps


def assemble_out(results, t_steps: int = T):
    out = np.empty((B_FULL, t_steps, 2 * H), np.float32)
    for idx, r in enumerate(results):
        d, g = divmod(idx, 4)
        h = r["y"].reshape(H, t_steps, B_LOC).transpose(2, 1, 0)  # [16, T, 512]
        if d == 1:
            h = h[:, ::-1, :]
        out[g * B_LOC:(g + 1) * B_LOC, :, d * H:(d + 1) * H] = h
    return out


def kernel(x, W_fw, b_fw, W_bw, b_bw):
    from concourse.bass_utils import run_bass_kernel_spmd

    nc = _get_built()
    in_maps = make_in_maps(x, W_fw, b_fw, W_bw, b_bw)
    res = run_bass_kernel_spmd(nc, in_maps, core_ids=list(range(8)))
    return assemble_out(res.results)

